# revision 15
# baseline (speedup 1.0000x reference)
"""Trainium2 Bass kernel for 2D cubic Hermite interpolation (nn_CubicHermite2d).

Math: with x1 = arange(W), x2 = arange(H) (per the problem spec), the whole
op is linear in `signal`:

    result[b, r, q] = sum_{h,w} M2[h, r] * signal[b, h, w] * M1[w, q]

where M1 [W, Nx] / M2 [H, Ny] are 4-banded cubic-Hermite interpolation
matrices built on the host from xs / ys.  Queries are sorted, so contiguous
query groups have source-row bands inside one of five fixed 128-row windows
(starts 0,96,...,384) -> every output block is a single K=128 matmul on the
PE (no accumulation, no transposes):

    step 1:  v[j1][wp, r]  = sig[win(g2), 96*j1:+128].T @ M2[win(g2), rs:re]
    step 2:  out[b, rm, q] = v[j1(g)][:, rm*128:+128].T @ M1[96*j1:+128, qs:qe]

The fixed window grid makes each batch's signal a single 3D strided DMA.
Matmuls run in float32r (single-pass fp32, ~2 cyc/row); fp32r requires even
matmul N and 8B-aligned PSUM offsets, so groups keep even sizes.  If the
query distribution ever defeats the even-size/window rules, the build falls
back to exact (4 cyc/row) float32.

Sharding: data-parallel over batch B=32 across 8 cores (4 batches/core).
"""

import os
import sys

import numpy as np

for _p in ("/root/.axon_site", "/root/.axon_site/_ro/trn_rl_repo",
           "/root/.axon_site/_ro/pypackages", "/opt/trn_rl_repo"):
    if os.path.isdir(_p) and _p not in sys.path:
        sys.path.append(_p)

import concourse.bass as bass
import concourse.mybir as mybir
from concourse import bacc
from concourse.bass_utils import run_bass_kernel_spmd
from concourse.tile import TileContext

# Problem shapes (hardcoded per spec)
B, H, W = 32, 512, 512
NX, NY = 1024, 1024
N_CORES = 8
NB = B // N_CORES  # batches per core

P = 128
WIN_STRIDE = 96          # window starts 0, 96, 192, 288, 384; each 128 rows
N_WIN = 5
F32 = mybir.dt.float32
USE_F32R = os.environ.get("CH2D_F32R", "1") == "1"


def _interp_matrix(n, u):
    """[n, Q] float64 matrix M with (y @ M) == cubic-Hermite interp of y at u,
    for grid x = arange(n), matching the reference's searchsorted/slope rules."""
    q = len(u)
    m = np.zeros((n, q), dtype=np.float64)
    idx = np.searchsorted(np.arange(1, n - 1, dtype=np.float64), u.astype(np.float64))
    t = u.astype(np.float64) - idx
    t2, t3 = t * t, t * t * t
    h00 = 1.0 - 3.0 * t2 + 2.0 * t3
    h10 = t - 2.0 * t2 + t3
    h01 = 3.0 * t2 - 2.0 * t3
    h11 = t3 - t2
    for k in range(q):
        i = int(idx[k])
        m[i, k] += h00[k]
        m[i + 1, k] += h01[k]
        if i == 0:
            m[1, k] += h10[k]
            m[0, k] -= h10[k]
        else:
            m[i + 1, k] += h10[k] / 2
            m[i - 1, k] -= h10[k] / 2
        if i + 1 == n - 1:
            m[n - 1, k] += h11[k]
            m[n - 2, k] -= h11[k]
        else:
            m[i + 2, k] += h11[k] / 2
            m[i, k] -= h11[k] / 2
    return m, idx.astype(np.int64)


def _make_groups(idx, n, bank=512):
    """Contiguous query groups assigned to fixed 128-row windows on the
    WIN_STRIDE grid.  Groups break at window changes and `bank`-multiples
    (PSUM bank boundary), keep sizes even (fp32r ISA needs even matmul N and
    8B-aligned PSUM column offsets), and never exceed `bank` queries.
    Returns ([(q_start, q_end, win_idx)], f32r_ok)."""
    qn = len(idx)
    lo = np.maximum(idx - 1, 0)
    hi = np.minimum(idx + 2, n - 1)
    win = np.minimum(lo // WIN_STRIDE, N_WIN - 1)

    def fits(k, j):
        return lo[k] >= WIN_STRIDE * j and hi[k] < WIN_STRIDE * j + P

    groups = []
    f32r_ok = True
    s = 0
    while s < qn:
        j = int(win[s])
        if not fits(s, j):  # defensive; shouldn't happen on this grid
            f32r_ok = False
        e = s
        while e < qn and win[e] == j and e - s < bank and not (e > s and e % bank == 0):
            e += 1
        if (e - s) % 2 == 1:
            if e < qn and fits(e, j) and not (e % bank == 0):
                e += 1  # steal the next query into this window to stay even
            else:
                f32r_ok = False
        groups.append((s, e, j))
        s = e
    return groups, f32r_ok


def _build_nc(g1, g2, mm_dt):
    MM_DT = mm_dt
    nc = bacc.Bacc("TRN2", target_bir_lowering=False,
                   name="cubic_hermite2d", num_devices=N_CORES)
    sig_d = nc.dram_tensor("signal", [NB, H, W], MM_DT, kind="ExternalInput")
    w2_d = nc.dram_tensor("w2p", [P, NY], MM_DT, kind="ExternalInput")
    w1_d = nc.dram_tensor("w1p", [P, NX], MM_DT, kind="ExternalInput")
    out_d = nc.dram_tensor("out", [NB, NY, NX], F32, kind="ExternalOutput")

    j1_list = sorted({g[2] for g in g1})  # windows that actually have queries
    copy_i = 0
    # per-bank halves so PSUM tiles are single-bank
    half1 = [[g for g in g1 if g[1] <= NX // 2], [g for g in g1 if g[0] >= NX // 2]]
    half2 = [[g for g in g2 if g[1] <= NY // 2], [g for g in g2 if g[0] >= NY // 2]]
    assert sum(map(len, half1)) == len(g1) and sum(map(len, half2)) == len(g2)

    with (
        TileContext(nc) as tc,
        tc.tile_pool(name="const", bufs=1) as const_pool,
        tc.tile_pool(name="sig", bufs=NB) as sig_pool,
        tc.tile_pool(name="vbuf", bufs=2 * len(j1_list)) as v_pool,
        tc.tile_pool(name="obuf", bufs=6) as o_pool,
        tc.tile_pool(name="vps", bufs=4, space="PSUM") as vps_pool,
        tc.tile_pool(name="ops", bufs=4, space="PSUM") as ops_pool,
    ):
        w2_s = const_pool.tile([P, NY], MM_DT, name="w2s")
        nc.sync.dma_start(out=w2_s[:], in_=w2_d[:, :])
        w1_s = const_pool.tile([P, NX], MM_DT, name="w1s")
        nc.sync.dma_start(out=w1_s[:], in_=w1_d[:, :])

        def copy_out(dst, src):
            # alternate PSUM->SBUF copies between DVE and ACT to split the load
            nonlocal copy_i
            if copy_i % 2 == 0:
                nc.vector.tensor_copy(out=dst, in_=src)
            else:
                nc.scalar.copy(out=dst, in_=src)
            copy_i += 1

        # preload all batches' signal windows: one 3D strided DMA per batch
        # dst [128, N_WIN, W]; src (p, j, w) = signal[b, WIN_STRIDE*j + p, w]
        sig_tiles = []
        for b in range(NB):
            st = sig_pool.tile([P, N_WIN, W], MM_DT, name="sigt")
            src = bass.AP(tensor=sig_d, offset=b * H * W,
                          ap=[[W, P], [WIN_STRIDE * W, N_WIN], [1, W]])
            nc.sync.dma_start(out=st[:], in_=src)
            sig_tiles.append(st)

        for b in range(NB):
            v_tiles = {}
            for j1 in j1_list:
                wlo = WIN_STRIDE * j1
                vt = v_pool.tile([P, NY], MM_DT, name="vt")
                for hi_, hgroups in enumerate(half2):
                    if not hgroups:
                        continue
                    base = hi_ * (NY // 2)
                    vps = vps_pool.tile([P, NY // 2], F32, name="vps")
                    for (rs, re, j2) in hgroups:
                        nc.tensor.matmul(
                            out=vps[:, rs - base:re - base],
                            lhsT=sig_tiles[b][:, j2, wlo:wlo + P],
                            rhs=w2_s[:, rs:re],
                            start=True, stop=True,
                        )
                    copy_out(vt[:, base:base + NY // 2], vps[:])
                v_tiles[j1] = vt

            # step 2: pairs of r-blocks share one staging tile and one store
            for mp in range(NY // P // 2):
                ot = o_pool.tile([P, 2 * NX], F32, name="ot")
                for sub in range(2):
                    mi = mp * 2 + sub
                    for hi_, hgroups in enumerate(half1):
                        if not hgroups:
                            continue
                        base = hi_ * (NX // 2)
                        ops = ops_pool.tile([P, NX // 2], F32, name="ops")
                        for (qs, qe, j1) in hgroups:
                            nc.tensor.matmul(
                                out=ops[:, qs - base:qe - base],
                                lhsT=v_tiles[j1][:, mi * P:(mi + 1) * P],
                                rhs=w1_s[:, qs:qe],
                                start=True, stop=True,
                            )
                        copy_out(ot[:, sub * NX + base:sub * NX + base + NX // 2],
                                 ops[:])
                dst = bass.AP(tensor=out_d,
                              offset=b * NY * NX + (mp * 2) * P * NX,
                              ap=[[NX, P], [P * NX, 2], [1, NX]])
                nc.sync.dma_start(out=dst, in_=ot[:])

    nc.compile()
    return nc


def _prepare(signal, x1, x2, xs, ys):
    """Host-side prep: sorted-order permutations, interp matrices, groups."""
    xs = np.asarray(xs, dtype=np.float32)
    ys = np.asarray(ys, dtype=np.float32)
    perm_x = None
    if np.any(np.diff(xs) < 0):
        perm_x = np.argsort(xs, kind="stable")
        xs = xs[perm_x]
    perm_y = None
    if np.any(np.diff(ys) < 0):
        perm_y = np.argsort(ys, kind="stable")
        ys = ys[perm_y]

    m1, i1 = _interp_matrix(W, xs)
    m2, i2 = _interp_matrix(H, ys)
    g1, ok1 = _make_groups(i1, W)
    g2, ok2 = _make_groups(i2, H)

    # pack band blocks: rows = the group's 128-row source window
    w1p = np.zeros((P, NX), dtype=np.float32)
    for (qs, qe, j) in g1:
        w1p[:, qs:qe] = m1[WIN_STRIDE * j:WIN_STRIDE * j + P, qs:qe]
    w2p = np.zeros((P, NY), dtype=np.float32)
    for (rs, re, j) in g2:
        w2p[:, rs:re] = m2[WIN_STRIDE * j:WIN_STRIDE * j + P, rs:re]
    return g1, g2, ok1 and ok2, w1p, w2p, perm_x, perm_y


_NC_CACHE = {}


def _run(inputs, trace=False, trace_kwargs=None):
    signal = np.ascontiguousarray(np.asarray(inputs["signal"], dtype=np.float32))
    g1, g2, f32r_ok, w1p, w2p, perm_x, perm_y = _prepare(
        signal, inputs["x1"], inputs["x2"], inputs["xs"], inputs["ys"])

    use_f32r = USE_F32R and f32r_ok
    mm_dt = mybir.dt.float32r if use_f32r else mybir.dt.float32
    key = (tuple(g1), tuple(g2), mm_dt)
    nc = _NC_CACHE.get(key)
    if nc is None:
        nc = _build_nc(g1, g2, mm_dt)
        _NC_CACHE[key] = nc

    in_maps = []
    for c in range(N_CORES):
        in_maps.append({
            "signal": np.ascontiguousarray(signal[c * NB:(c + 1) * NB]),
            "w2p": w2p,
            "w1p": w1p,
        })
    res = run_bass_kernel_spmd(
        nc, in_maps, core_ids=list(range(N_CORES)),
        trace=trace, **(trace_kwargs or {}),
    )
    out = np.concatenate([r["out"] for r in res.results], axis=0)

    # restore original (unsorted) query order if needed
    if perm_y is not None:
        inv = np.empty_like(perm_y)
        inv[perm_y] = np.arange(len(perm_y))
        out = out[:, inv, :]
    if perm_x is not None:
        inv = np.empty_like(perm_x)
        inv[perm_x] = np.arange(len(perm_x))
        out = out[:, :, inv]
    return out, res


def kernel(signal, x1, x2, xs, ys):
    out, _ = _run({"signal": signal, "x1": x1, "x2": x2, "xs": xs, "ys": ys})
    return out


# revision 19
# speedup vs baseline: 1.1677x; 1.1677x over previous
"""Trainium2 Bass kernel for 2D cubic Hermite interpolation (nn_CubicHermite2d).

Math: with x1 = arange(W), x2 = arange(H) (per the problem spec), the whole
op is linear in `signal`:

    result[b, r, q] = sum_{h,w} M2[h, r] * signal[b, h, w] * M1[w, q]

where M1 [W, Nx] / M2 [H, Ny] are 4-banded cubic-Hermite interpolation
matrices built on the host from xs / ys.  Queries are sorted, so contiguous
query groups have source-row bands inside one of five fixed 128-row windows
(starts 0,96,...,384) -> every output block is a single K=128 matmul on the
PE (no accumulation, no transposes):

    step 1:  v[j1][wp, r]  = sig[win(g2), 96*j1:+128].T @ M2[win(g2), rs:re]
    step 2:  out[b, rm, q] = v[j1(g)][:, rm*128:+128].T @ M1[96*j1:+128, qs:qe]

The fixed window grid makes each batch's signal a single 3D strided DMA.
Matmuls run in float32r (single-pass fp32, ~2 cyc/row); fp32r requires even
matmul N and 8B-aligned PSUM offsets, so groups keep even sizes.  If the
query distribution ever defeats the even-size/window rules, the build falls
back to exact (4 cyc/row) float32.

Sharding: data-parallel over batch B=32 across 8 cores (4 batches/core).
"""

import os
import sys

import numpy as np

for _p in ("/root/.axon_site", "/root/.axon_site/_ro/trn_rl_repo",
           "/root/.axon_site/_ro/pypackages", "/opt/trn_rl_repo"):
    if os.path.isdir(_p) and _p not in sys.path:
        sys.path.append(_p)

import concourse.bass as bass
import concourse.mybir as mybir
from concourse import bacc
from concourse.bass_utils import run_bass_kernel_spmd
from concourse.tile import TileContext

# Problem shapes (hardcoded per spec)
B, H, W = 32, 512, 512
NX, NY = 1024, 1024
N_CORES = 8
NB = B // N_CORES  # batches per core

P = 128
WIN_STRIDE = 96          # window starts 0, 96, 192, 288, 384; each 128 rows
N_WIN = 5
F32 = mybir.dt.float32
USE_F32R = os.environ.get("CH2D_F32R", "1") == "1"


def _interp_matrix(n, u):
    """[n, Q] float64 matrix M with (y @ M) == cubic-Hermite interp of y at u,
    for grid x = arange(n), matching the reference's searchsorted/slope rules."""
    q = len(u)
    m = np.zeros((n, q), dtype=np.float64)
    idx = np.searchsorted(np.arange(1, n - 1, dtype=np.float64), u.astype(np.float64))
    t = u.astype(np.float64) - idx
    t2, t3 = t * t, t * t * t
    h00 = 1.0 - 3.0 * t2 + 2.0 * t3
    h10 = t - 2.0 * t2 + t3
    h01 = 3.0 * t2 - 2.0 * t3
    h11 = t3 - t2
    for k in range(q):
        i = int(idx[k])
        m[i, k] += h00[k]
        m[i + 1, k] += h01[k]
        if i == 0:
            m[1, k] += h10[k]
            m[0, k] -= h10[k]
        else:
            m[i + 1, k] += h10[k] / 2
            m[i - 1, k] -= h10[k] / 2
        if i + 1 == n - 1:
            m[n - 1, k] += h11[k]
            m[n - 2, k] -= h11[k]
        else:
            m[i + 2, k] += h11[k] / 2
            m[i, k] -= h11[k] / 2
    return m, idx.astype(np.int64)


def _make_groups(idx, n, max_size=512, bank=512):
    """Greedy contiguous query groups; each group's source rows fit a
    128-row window starting at row_lo.  Groups never cross `bank`-multiples
    in query index (PSUM bank boundary) and keep even sizes where possible
    (fp32r ISA needs even matmul N and 8B-aligned PSUM column offsets).
    Returns ([(q_start, q_end, row_lo)], f32r_ok)."""
    qn = len(idx)
    lo = np.maximum(idx - 1, 0)
    hi = np.minimum(idx + 2, n - 1)
    groups = []
    s = 0
    while s < qn:
        row_lo = int(lo[s])
        e = s
        while e < qn:
            if hi[e] - row_lo + 1 > P:
                break
            if e - s >= max_size:
                break
            if e > s and (e % bank) == 0:
                break
            e += 1
        if e < qn and (e - s) % 2 == 1 and e - s > 1:
            e -= 1  # keep sizes (and hence starts) even for fp32r
        groups.append((s, e, min(row_lo, n - P)))
        s = e
    f32r_ok = all(q % 2 == 0 and (e - q) % 2 == 0 for q, e, _ in groups)
    return groups, f32r_ok


def _build_nc(g1, g2, mm_dt):
    MM_DT = mm_dt
    nc = bacc.Bacc("TRN2", target_bir_lowering=False,
                   name="cubic_hermite2d", num_devices=N_CORES)
    sig_d = nc.dram_tensor("signal", [NB, H, W], MM_DT, kind="ExternalInput")
    w2_d = nc.dram_tensor("w2p", [P, NY], MM_DT, kind="ExternalInput")
    w1_d = nc.dram_tensor("w1p", [P, NX], MM_DT, kind="ExternalInput")
    out_d = nc.dram_tensor("out", [NB, NY, NX], F32, kind="ExternalOutput")

    wlo1_list = sorted({g[2] for g in g1})  # distinct xs source windows
    wlo2_list = sorted({g[2] for g in g2})  # distinct ys source windows
    copy_i = 0
    # per-bank halves so PSUM tiles are single-bank
    half1 = [[g for g in g1 if g[1] <= NX // 2], [g for g in g1 if g[0] >= NX // 2]]
    half2 = [[g for g in g2 if g[1] <= NY // 2], [g for g in g2 if g[0] >= NY // 2]]
    assert sum(map(len, half1)) == len(g1) and sum(map(len, half2)) == len(g2)

    with (
        TileContext(nc) as tc,
        tc.tile_pool(name="const", bufs=1) as const_pool,
        tc.tile_pool(name="sig", bufs=len(wlo2_list)) as sig_pool,
        tc.tile_pool(name="vbuf", bufs=2 * len(wlo1_list)) as v_pool,
        tc.tile_pool(name="obuf", bufs=6) as o_pool,
        tc.tile_pool(name="vps", bufs=4, space="PSUM") as vps_pool,
        tc.tile_pool(name="ops", bufs=4, space="PSUM") as ops_pool,
    ):
        w2_s = const_pool.tile([P, NY], MM_DT, name="w2s")
        nc.sync.dma_start(out=w2_s[:], in_=w2_d[:, :])
        w1_s = const_pool.tile([P, NX], MM_DT, name="w1s")
        nc.sync.dma_start(out=w1_s[:], in_=w1_d[:, :])

        def copy_out(dst, src):
            # alternate PSUM->SBUF copies between DVE and ACT to split the load
            nonlocal copy_i
            if copy_i % 2 == 0:
                nc.vector.tensor_copy(out=dst, in_=src)
            else:
                nc.scalar.copy(out=dst, in_=src)
            copy_i += 1

        # preload signal: one 3D strided DMA per source window covering ALL
        # batches; dst [128, NB, W]; src (p, b, w) = signal[b, hlo + p, w]
        sig_tiles = {}
        for hlo in wlo2_list:
            st = sig_pool.tile([P, NB, W], MM_DT, name="sigt")
            src = bass.AP(tensor=sig_d, offset=hlo * W,
                          ap=[[W, P], [H * W, NB], [1, W]])
            nc.sync.dma_start(out=st[:], in_=src)
            sig_tiles[hlo] = st

        for b in range(NB):
            v_tiles = {}
            for wlo in wlo1_list:
                vt = v_pool.tile([P, NY], MM_DT, name="vt")
                for hi_, hgroups in enumerate(half2):
                    if not hgroups:
                        continue
                    base = hi_ * (NY // 2)
                    vps = vps_pool.tile([P, NY // 2], F32, name="vps")
                    for (rs, re, hlo) in hgroups:
                        nc.tensor.matmul(
                            out=vps[:, rs - base:re - base],
                            lhsT=sig_tiles[hlo][:, b, wlo:wlo + P],
                            rhs=w2_s[:, rs:re],
                            start=True, stop=True,
                        )
                    copy_out(vt[:, base:base + NY // 2], vps[:])
                v_tiles[wlo] = vt

            # step 2: pairs of r-blocks share one staging tile and one store
            for mp in range(NY // P // 2):
                ot = o_pool.tile([P, 2 * NX], F32, name="ot")
                for sub in range(2):
                    mi = mp * 2 + sub
                    for hi_, hgroups in enumerate(half1):
                        if not hgroups:
                            continue
                        base = hi_ * (NX // 2)
                        ops = ops_pool.tile([P, NX // 2], F32, name="ops")
                        for (qs, qe, wlo) in hgroups:
                            nc.tensor.matmul(
                                out=ops[:, qs - base:qe - base],
                                lhsT=v_tiles[wlo][:, mi * P:(mi + 1) * P],
                                rhs=w1_s[:, qs:qe],
                                start=True, stop=True,
                            )
                        copy_out(ot[:, sub * NX + base:sub * NX + base + NX // 2],
                                 ops[:])
                dst = bass.AP(tensor=out_d,
                              offset=b * NY * NX + (mp * 2) * P * NX,
                              ap=[[NX, P], [P * NX, 2], [1, NX]])
                nc.sync.dma_start(out=dst, in_=ot[:])

    nc.compile()
    return nc


def _prepare(signal, x1, x2, xs, ys):
    """Host-side prep: sorted-order permutations, interp matrices, groups."""
    xs = np.asarray(xs, dtype=np.float32)
    ys = np.asarray(ys, dtype=np.float32)
    perm_x = None
    if np.any(np.diff(xs) < 0):
        perm_x = np.argsort(xs, kind="stable")
        xs = xs[perm_x]
    perm_y = None
    if np.any(np.diff(ys) < 0):
        perm_y = np.argsort(ys, kind="stable")
        ys = ys[perm_y]

    m1, i1 = _interp_matrix(W, xs)
    m2, i2 = _interp_matrix(H, ys)
    g1, ok1 = _make_groups(i1, W)
    g2, ok2 = _make_groups(i2, H)

    # pack band blocks: rows = the group's 128-row source window
    w1p = np.zeros((P, NX), dtype=np.float32)
    for (qs, qe, wlo) in g1:
        w1p[:, qs:qe] = m1[wlo:wlo + P, qs:qe]
    w2p = np.zeros((P, NY), dtype=np.float32)
    for (rs, re, hlo) in g2:
        w2p[:, rs:re] = m2[hlo:hlo + P, rs:re]
    return g1, g2, ok1 and ok2, w1p, w2p, perm_x, perm_y


_NC_CACHE = {}


def _run(inputs, trace=False, trace_kwargs=None):
    signal = np.ascontiguousarray(np.asarray(inputs["signal"], dtype=np.float32))
    g1, g2, f32r_ok, w1p, w2p, perm_x, perm_y = _prepare(
        signal, inputs["x1"], inputs["x2"], inputs["xs"], inputs["ys"])

    use_f32r = USE_F32R and f32r_ok
    mm_dt = mybir.dt.float32r if use_f32r else mybir.dt.float32
    key = (tuple(g1), tuple(g2), mm_dt)
    nc = _NC_CACHE.get(key)
    if nc is None:
        nc = _build_nc(g1, g2, mm_dt)
        _NC_CACHE[key] = nc

    in_maps = []
    for c in range(N_CORES):
        in_maps.append({
            "signal": np.ascontiguousarray(signal[c * NB:(c + 1) * NB]),
            "w2p": w2p,
            "w1p": w1p,
        })
    res = run_bass_kernel_spmd(
        nc, in_maps, core_ids=list(range(N_CORES)),
        trace=trace, **(trace_kwargs or {}),
    )
    out = np.concatenate([r["out"] for r in res.results], axis=0)

    # restore original (unsorted) query order if needed
    if perm_y is not None:
        inv = np.empty_like(perm_y)
        inv[perm_y] = np.arange(len(perm_y))
        out = out[:, inv, :]
    if perm_x is not None:
        inv = np.empty_like(perm_x)
        inv[perm_x] = np.arange(len(perm_x))
        out = out[:, :, inv]
    return out, res


def kernel(signal, x1, x2, xs, ys):
    out, _ = _run({"signal": signal, "x1": x1, "x2": x2, "xs": xs, "ys": ys})
    return out


# revision 22
# speedup vs baseline: 1.3331x; 1.1416x over previous
"""Trainium2 Bass kernel for 2D cubic Hermite interpolation (nn_CubicHermite2d).

Math: with x1 = arange(W), x2 = arange(H) (per the problem spec), the whole
op is linear in `signal`:

    result[b, r, q] = sum_{h,w} M2[h, r] * signal[b, h, w] * M1[w, q]

where M1 [W, Nx] / M2 [H, Ny] are 4-banded cubic-Hermite interpolation
matrices built on the host from xs / ys.  Queries are sorted, so contiguous
query groups have source-row bands inside one of five fixed 128-row windows
(starts 0,96,...,384) -> every output block is a single K=128 matmul on the
PE (no accumulation, no transposes):

    step 1:  v[j1][wp, r]  = sig[win(g2), 96*j1:+128].T @ M2[win(g2), rs:re]
    step 2:  out[b, rm, q] = v[j1(g)][:, rm*128:+128].T @ M1[96*j1:+128, qs:qe]

The fixed window grid makes each batch's signal a single 3D strided DMA.
Matmuls run in float32r (single-pass fp32, ~2 cyc/row); fp32r requires even
matmul N and 8B-aligned PSUM offsets, so groups keep even sizes.  If the
query distribution ever defeats the even-size/window rules, the build falls
back to exact (4 cyc/row) float32.

Sharding: data-parallel over batch B=32 across 8 cores (4 batches/core).
"""

import os
import sys

import numpy as np

for _p in ("/root/.axon_site", "/root/.axon_site/_ro/trn_rl_repo",
           "/root/.axon_site/_ro/pypackages", "/opt/trn_rl_repo"):
    if os.path.isdir(_p) and _p not in sys.path:
        sys.path.append(_p)

import concourse.bass as bass
import concourse.mybir as mybir
from concourse import bacc
from concourse.bass_utils import run_bass_kernel_spmd
from concourse.tile import TileContext

# Problem shapes (hardcoded per spec)
B, H, W = 32, 512, 512
NX, NY = 1024, 1024
N_CORES = 8
NB = B // N_CORES  # batches per core

P = 128
WIN_STRIDE = 96          # window starts 0, 96, 192, 288, 384; each 128 rows
N_WIN = 5
F32 = mybir.dt.float32
USE_F32R = os.environ.get("CH2D_F32R", "1") == "1"


def _interp_matrix(n, u):
    """[n, Q] float64 matrix M with (y @ M) == cubic-Hermite interp of y at u,
    for grid x = arange(n), matching the reference's searchsorted/slope rules."""
    q = len(u)
    m = np.zeros((n, q), dtype=np.float64)
    idx = np.searchsorted(np.arange(1, n - 1, dtype=np.float64), u.astype(np.float64))
    t = u.astype(np.float64) - idx
    t2, t3 = t * t, t * t * t
    h00 = 1.0 - 3.0 * t2 + 2.0 * t3
    h10 = t - 2.0 * t2 + t3
    h01 = 3.0 * t2 - 2.0 * t3
    h11 = t3 - t2
    for k in range(q):
        i = int(idx[k])
        m[i, k] += h00[k]
        m[i + 1, k] += h01[k]
        if i == 0:
            m[1, k] += h10[k]
            m[0, k] -= h10[k]
        else:
            m[i + 1, k] += h10[k] / 2
            m[i - 1, k] -= h10[k] / 2
        if i + 1 == n - 1:
            m[n - 1, k] += h11[k]
            m[n - 2, k] -= h11[k]
        else:
            m[i + 2, k] += h11[k] / 2
            m[i, k] -= h11[k] / 2
    return m, idx.astype(np.int64)


def _make_groups(idx, n, max_size=512, bank=512):
    """Greedy contiguous query groups; each group's source rows fit a
    128-row window starting at row_lo.  Groups never cross `bank`-multiples
    in query index (PSUM bank boundary) and keep even sizes where possible
    (fp32r ISA needs even matmul N and 8B-aligned PSUM column offsets).
    Returns ([(q_start, q_end, row_lo)], f32r_ok)."""
    qn = len(idx)
    lo = np.maximum(idx - 1, 0)
    hi = np.minimum(idx + 2, n - 1)
    groups = []
    s = 0
    while s < qn:
        row_lo = int(lo[s])
        e = s
        while e < qn:
            if hi[e] - row_lo + 1 > P:
                break
            if e - s >= max_size:
                break
            if e > s and (e % bank) == 0:
                break
            e += 1
        if e < qn and (e - s) % 2 == 1 and e - s > 1:
            e -= 1  # keep sizes (and hence starts) even for fp32r
        groups.append((s, e, min(row_lo, n - P)))
        s = e
    f32r_ok = all(q % 2 == 0 and (e - q) % 2 == 0 for q, e, _ in groups)
    return groups, f32r_ok


def _build_nc(g1, g2, mm_dt):
    MM_DT = mm_dt
    nc = bacc.Bacc("TRN2", target_bir_lowering=False,
                   name="cubic_hermite2d", num_devices=N_CORES)
    sig_d = nc.dram_tensor("signal", [NB, H, W], MM_DT, kind="ExternalInput")
    w2_d = nc.dram_tensor("w2p", [P, NY], MM_DT, kind="ExternalInput")
    w1_d = nc.dram_tensor("w1p", [P, NX], MM_DT, kind="ExternalInput")
    out_d = nc.dram_tensor("out", [NB, NY, NX], F32, kind="ExternalOutput")

    wlo1_list = sorted({g[2] for g in g1})  # distinct xs source windows
    wlo2_list = sorted({g[2] for g in g2})  # distinct ys source windows
    copy_i = 0
    # per-bank halves so PSUM tiles are single-bank
    half1 = [[g for g in g1 if g[1] <= NX // 2], [g for g in g1 if g[0] >= NX // 2]]
    half2 = [[g for g in g2 if g[1] <= NY // 2], [g for g in g2 if g[0] >= NY // 2]]
    assert sum(map(len, half1)) == len(g1) and sum(map(len, half2)) == len(g2)

    with (
        TileContext(nc) as tc,
        tc.tile_pool(name="const", bufs=1) as const_pool,
        tc.tile_pool(name="sig", bufs=len(wlo2_list)) as sig_pool,
        tc.tile_pool(name="vbuf", bufs=2 * len(wlo1_list)) as v_pool,
        tc.tile_pool(name="obuf", bufs=6) as o_pool,
        tc.tile_pool(name="vps", bufs=4, space="PSUM") as vps_pool,
        tc.tile_pool(name="ops", bufs=4, space="PSUM") as ops_pool,
    ):
        w2_s = const_pool.tile([P, NY], MM_DT, name="w2s")
        nc.sync.dma_start(out=w2_s[:], in_=w2_d[:, :])

        def copy_out(dst, src):
            # alternate PSUM->SBUF copies between DVE and ACT to split the load
            nonlocal copy_i
            if copy_i % 2 == 0:
                nc.vector.tensor_copy(out=dst, in_=src)
            else:
                nc.scalar.copy(out=dst, in_=src)
            copy_i += 1

        # preload signal: per source window, batch 0 first (small, unblocks
        # the first matmuls fast), then batches 1..NB-1 in one strided DMA
        # dst [128, NB, W]; src (p, b, w) = signal[b, hlo + p, w]
        sig_tiles = {}
        for hlo in wlo2_list:
            st = sig_pool.tile([P, NB, W], MM_DT, name="sigt")
            nc.sync.dma_start(out=st[:, 0, :], in_=sig_d[0, hlo:hlo + P, :])
            sig_tiles[hlo] = st
        w1_s = const_pool.tile([P, NX], MM_DT, name="w1s")
        nc.sync.dma_start(out=w1_s[:], in_=w1_d[:, :])
        for hlo in wlo2_list:
            src = bass.AP(tensor=sig_d, offset=H * W + hlo * W,
                          ap=[[W, P], [H * W, NB - 1], [1, W]])
            nc.sync.dma_start(out=sig_tiles[hlo][:, 1:, :], in_=src)

        for b in range(NB):
            v_tiles = {}
            for wlo in wlo1_list:
                vt = v_pool.tile([P, NY], MM_DT, name="vt")
                for hi_, hgroups in enumerate(half2):
                    if not hgroups:
                        continue
                    base = hi_ * (NY // 2)
                    vps = vps_pool.tile([P, NY // 2], F32, name="vps")
                    for (rs, re, hlo) in hgroups:
                        nc.tensor.matmul(
                            out=vps[:, rs - base:re - base],
                            lhsT=sig_tiles[hlo][:, b, wlo:wlo + P],
                            rhs=w2_s[:, rs:re],
                            start=True, stop=True,
                        )
                    copy_out(vt[:, base:base + NY // 2], vps[:])
                v_tiles[wlo] = vt

            # step 2: pairs of r-blocks share one staging tile and one store;
            # the final batch stores per-block so its tail drains sooner
            pair = 1 if b == NB - 1 else 2
            for mp in range(NY // P // pair):
                ot = o_pool.tile([P, pair * NX], F32, name="ot",
                                 padded_shape=[P, 2 * NX])
                for sub in range(pair):
                    mi = mp * pair + sub
                    for hi_, hgroups in enumerate(half1):
                        if not hgroups:
                            continue
                        base = hi_ * (NX // 2)
                        ops = ops_pool.tile([P, NX // 2], F32, name="ops")
                        for (qs, qe, wlo) in hgroups:
                            nc.tensor.matmul(
                                out=ops[:, qs - base:qe - base],
                                lhsT=v_tiles[wlo][:, mi * P:(mi + 1) * P],
                                rhs=w1_s[:, qs:qe],
                                start=True, stop=True,
                            )
                        copy_out(ot[:, sub * NX + base:sub * NX + base + NX // 2],
                                 ops[:])
                dst = bass.AP(tensor=out_d,
                              offset=b * NY * NX + (mp * pair) * P * NX,
                              ap=[[NX, P], [P * NX, pair], [1, NX]])
                nc.sync.dma_start(out=dst, in_=ot[:])

    nc.compile()
    return nc


def _prepare(signal, x1, x2, xs, ys):
    """Host-side prep: sorted-order permutations, interp matrices, groups."""
    xs = np.asarray(xs, dtype=np.float32)
    ys = np.asarray(ys, dtype=np.float32)
    perm_x = None
    if np.any(np.diff(xs) < 0):
        perm_x = np.argsort(xs, kind="stable")
        xs = xs[perm_x]
    perm_y = None
    if np.any(np.diff(ys) < 0):
        perm_y = np.argsort(ys, kind="stable")
        ys = ys[perm_y]

    m1, i1 = _interp_matrix(W, xs)
    m2, i2 = _interp_matrix(H, ys)
    g1, ok1 = _make_groups(i1, W)
    g2, ok2 = _make_groups(i2, H)

    # pack band blocks: rows = the group's 128-row source window
    w1p = np.zeros((P, NX), dtype=np.float32)
    for (qs, qe, wlo) in g1:
        w1p[:, qs:qe] = m1[wlo:wlo + P, qs:qe]
    w2p = np.zeros((P, NY), dtype=np.float32)
    for (rs, re, hlo) in g2:
        w2p[:, rs:re] = m2[hlo:hlo + P, rs:re]
    return g1, g2, ok1 and ok2, w1p, w2p, perm_x, perm_y


_NC_CACHE = {}


def _run(inputs, trace=False, trace_kwargs=None):
    signal = np.ascontiguousarray(np.asarray(inputs["signal"], dtype=np.float32))
    g1, g2, f32r_ok, w1p, w2p, perm_x, perm_y = _prepare(
        signal, inputs["x1"], inputs["x2"], inputs["xs"], inputs["ys"])

    use_f32r = USE_F32R and f32r_ok
    mm_dt = mybir.dt.float32r if use_f32r else mybir.dt.float32
    key = (tuple(g1), tuple(g2), mm_dt)
    nc = _NC_CACHE.get(key)
    if nc is None:
        nc = _build_nc(g1, g2, mm_dt)
        _NC_CACHE[key] = nc

    in_maps = []
    for c in range(N_CORES):
        in_maps.append({
            "signal": np.ascontiguousarray(signal[c * NB:(c + 1) * NB]),
            "w2p": w2p,
            "w1p": w1p,
        })
    res = run_bass_kernel_spmd(
        nc, in_maps, core_ids=list(range(N_CORES)),
        trace=trace, **(trace_kwargs or {}),
    )
    out = np.concatenate([r["out"] for r in res.results], axis=0)

    # restore original (unsorted) query order if needed
    if perm_y is not None:
        inv = np.empty_like(perm_y)
        inv[perm_y] = np.arange(len(perm_y))
        out = out[:, inv, :]
    if perm_x is not None:
        inv = np.empty_like(perm_x)
        inv[perm_x] = np.arange(len(perm_x))
        out = out[:, :, inv]
    return out, res


def kernel(signal, x1, x2, xs, ys):
    out, _ = _run({"signal": signal, "x1": x1, "x2": x2, "xs": xs, "ys": ys})
    return out
